# revision 40
# baseline (speedup 1.0000x reference)
"""Trainium2 Bass kernel for nn_ControlFlexHNN (dense_mlp).

Data-parallel across 8 NeuronCores: batch N=32768 -> 4096 rows/core.
Activations are feature-major on-chip ([feature, batch]); every matmul
contracts over the partition dimension.

v3 design (vs f32r baseline):
  - f32r matmul operands everywhere (bf16 was 6x slower on HW: every
    2-byte matmul needs a separate InstLdweights, ~1.1us extra each;
    f32r matmuls self-load their stationary operand).
  - Wh folded into W2 host-side (W2W = W2 * Wh[:,None]) so the backward
    matmul contracts s1 directly - no ga2 array or scaling op.
  - Biases of the small (contract 16/20) layers folded into the matmul
    via a ones-row in zu ([z(16), 1, u(4)] layout).
  - j-chunks processed in pairs: elementwise ops run at width 1024,
    halving instruction count + fixed overheads.
  - Engine rebalance: ACT = tanh + square only, DVE = fused
    scalar_tensor_tensor / tensor_scalar, Pool = the two adds.
  - Stage-A matmuls+tanh of tile t+1 ride inside stage E of tile t
    (h1 double-buffered); s0 = 1-h1^2 is computed at the head of tile
    t+1 (s0 single-buffered bf16 - it is not a matmul operand).

Device kernel per core (B=512 batch tile, 8 tiles):
  A: pa1 = W1x @ [z;1]         h1 = tanh(pa1)         (rides in E(t-1))
  S: s0 = 1 - h1^2                                    (tile head)
  B: pb = W2 @ h1 (+b2)        s1 = 1-tanh^2(pb+b2)
  C: pf = Wf1x @ [z;1;u]       g1 = tanh(pf) + pf*s1
  D: pg = W2W.T @ s1           ga1 = pg*s0;  ps  = W1.T @ ga1   (dH)
  E: pf2 = Wf2 @ g1            g2 = tanh(pf2+bf2) + (pf2+bf2)*s0
                               ps += Wff @ g2;  out = ps + bff
"""

import numpy as np

N = 32768
DQ = 8
D2 = 2 * DQ          # 16
A_DIM = 4
Z1 = D2 + 1          # 17 rows: z + ones   (layer-A lhs contract)
ZU1 = D2 + 1 + A_DIM  # 21 rows: z + ones + u (layer-C lhs contract)
H = 1024
HC = H // 128        # 8 chunks
NP = 2               # chunks per elementwise pair
PAIRS = HC // NP     # 4
NCORES = 8
NSH = N // NCORES    # 4096 rows per core
B = 512              # batch tile (free dim of matmuls)
TILES = NSH // B     # 8

_BUILT = None


def _build(loop_n=None):
    """Build the kernel. loop_n wraps the whole 8-tile body in an on-device
    For_i loop (used only for HW timing via replication differencing)."""
    import contextlib

    import concourse.bacc as bacc
    import concourse.mybir as mybir
    from concourse import tile

    f32 = mybir.dt.float32
    f32r = mybir.dt.float32r

    nc = bacc.Bacc(None)

    zut_d = nc.dram_tensor("zut", [ZU1, NSH], f32r, kind="ExternalInput")
    w1t_d = nc.dram_tensor("w1t", [Z1, H], f32r, kind="ExternalInput")
    w1n_d = nc.dram_tensor("w1n", [H, D2], f32r, kind="ExternalInput")
    w2t_d = nc.dram_tensor("w2t", [H, H], f32r, kind="ExternalInput")
    w2wn_d = nc.dram_tensor("w2wn", [H, H], f32r, kind="ExternalInput")
    wf1t_d = nc.dram_tensor("wf1t", [ZU1, H], f32r, kind="ExternalInput")
    wf2t_d = nc.dram_tensor("wf2t", [H, H], f32r, kind="ExternalInput")
    wfft_d = nc.dram_tensor("wfft", [H, D2], f32r, kind="ExternalInput")
    b2c_d = nc.dram_tensor("b2c", [128, HC], f32, kind="ExternalInput")
    bf2c_d = nc.dram_tensor("bf2c", [128, HC], f32, kind="ExternalInput")
    bffc_d = nc.dram_tensor("bffc", [D2, 1], f32, kind="ExternalInput")
    st_d = nc.dram_tensor("st", [D2, NSH], f32, kind="ExternalOutput")

    with tile.TileContext(nc) as tc:
        with (
            tc.tile_pool(name="wp", bufs=1) as wp,
            tc.tile_pool(name="actp", bufs=1) as actp,
            tc.tile_pool(name="tmpp", bufs=1) as tmpp,
            tc.tile_pool(name="grp", bufs=2) as grp,
            tc.tile_pool(name="iop", bufs=2) as iop,
            tc.tile_pool(name="mmp", bufs=3, space="PSUM") as mmp,
            tc.tile_pool(name="accp", bufs=2, space="PSUM") as accp,
        ):
            # zut(0) first: everything in tile 0 hangs off it and the DMA
            # queue is in-order - behind 12MB of weights it would stall
            # the whole prologue.  (Looped timing builds re-DMA it in-body.)
            early_zut = None
            if not loop_n:
                early_zut = iop.tile([ZU1, B], f32r, tag="zut",
                                     name="zut_0")
                nc.sync.dma_start(early_zut[:], zut_d[:, 0:B])

            # ---- resident weights (DMA order = first-use order; the big
            # [H,H] weights stream per contraction chunk so tile-0 matmuls
            # can start as soon as their chunk lands) ----
            w1t = wp.tile([Z1, H], f32r)
            nc.sync.dma_start(w1t[:], w1t_d[:])
            wf1t = wp.tile([ZU1, H], f32r)
            nc.sync.dma_start(wf1t[:], wf1t_d[:])
            b2c = wp.tile([128, HC], f32)
            nc.sync.dma_start(b2c[:], b2c_d[:])
            w2t = wp.tile([128, HC, H], f32r)
            w2t_r = w2t_d.rearrange("(c p) j -> p c j", p=128)
            for cch in range(HC):
                nc.sync.dma_start(w2t[:, cch, :], w2t_r[:, cch, :])
            w2wn = wp.tile([128, HC, H], f32r)
            w2wn_r = w2wn_d.rearrange("(c p) k -> p c k", p=128)
            for cch in range(HC):
                nc.sync.dma_start(w2wn[:, cch, :], w2wn_r[:, cch, :])
            w1n = wp.tile([128, HC, D2], f32r)
            nc.sync.dma_start(w1n[:], w1n_d.rearrange("(c p) m -> p c m", p=128))
            wf2t = wp.tile([128, HC, H], f32r)
            wf2t_r = wf2t_d.rearrange("(c p) j -> p c j", p=128)
            for cch in range(HC):
                nc.sync.dma_start(wf2t[:, cch, :], wf2t_r[:, cch, :])
            bf2c = wp.tile([128, HC], f32)
            nc.sync.dma_start(bf2c[:], bf2c_d[:])
            wfft = wp.tile([128, HC, D2], f32r)
            nc.sync.dma_start(wfft[:], wfft_d.rearrange("(c p) m -> p c m", p=128))
            bffc = wp.tile([D2, 1], f32)
            nc.sync.dma_start(bffc[:], bffc_d[:])

            ws = (w1t, w1n, w2t, w2wn, wf1t, wf2t, wfft, b2c, bf2c, bffc)
            pools = (actp, tmpp, grp, iop, mmp, accp)

            loop_cm = tc.For_i(0, loop_n, 1) if loop_n else contextlib.nullcontext()
            with loop_cm:
                _emit_body(nc, mybir, pools, ws, zut_d, st_d, early_zut)

    nc.compile()
    return nc


def _build_looped(loop_n):
    return _build(loop_n=loop_n)


def _emit_A_mms(nc, mybir, mmp, w1t, zut, i, t):
    """Stage-A matmuls for chunk-pair i of tile t: pa1 = W1x @ [z;1]."""
    f32 = mybir.dt.float32
    pa = mmp.tile([128, NP, B], f32, tag="mm", name=f"pa_{t}_{i}")
    for h in range(NP):
        j = NP * i + h
        nc.tensor.matmul(pa[:, h, :], w1t[:, j * 128:(j + 1) * 128],
                         zut[0:Z1, :], start=True, stop=True)
    return pa


def _emit_A(nc, mybir, tmpp, mmp, w1t, zut, h1, i, t):
    """Stage A for chunk-pair i of tile t: pa1 = W1x @ [z;1]; h1 = tanh."""
    Tanh = mybir.ActivationFunctionType.Tanh
    pa = _emit_A_mms(nc, mybir, mmp, w1t, zut, i, t)
    nc.scalar.activation(h1[:, NP * i:NP * i + NP, :], pa[:], Tanh)


def _emit_body(nc, mybir, pools, ws, zut_d, st_d, early_zut=None):
    f32 = mybir.dt.float32
    f32r = mybir.dt.float32r
    bf16 = mybir.dt.bfloat16
    Tanh = mybir.ActivationFunctionType.Tanh
    Sigmoid = mybir.ActivationFunctionType.Sigmoid
    Ident = mybir.ActivationFunctionType.Identity
    mult = mybir.AluOpType.mult
    add = mybir.AluOpType.add

    actp, tmpp, grp, iop, mmp, accp = pools
    w1t, w1n, w2t, w2wn, wf1t, wf2t, wfft, b2c, bf2c, bffc = ws

    # ---- prologue: zut(0) + A(0) ----
    if early_zut is not None:
        zut = early_zut
    else:
        zut = iop.tile([ZU1, B], f32r, tag="zut", name="zut_0")
        nc.sync.dma_start(zut[:], zut_d[:, 0:B])
    h1 = actp.tile([128, HC, B], f32r, tag="h1", bufs=2, name="h1_0")
    for i in range(PAIRS):
        _emit_A(nc, mybir, tmpp, mmp, w1t, zut, h1, i, 0)

    tail = None  # deferred last F-head pair + sout of the previous tile

    for t in range(TILES):
        sl_t = slice(t * B, (t + 1) * B)
        # prefetch zut(t+1)
        if t + 1 < TILES:
            zut_n = iop.tile([ZU1, B], f32r, tag="zut", name=f"zut_{t + 1}")
            nc.sync.dma_start(zut_n[:], zut_d[:, (t + 1) * B:(t + 2) * B])
        else:
            zut_n = None

        # ---- S: s0 = 1 - h1^2 (bf16, not a matmul operand) ----
        # on Pool: same-engine in-order chain, keeps ACT free for the
        # B-stage sigmoid chain that gates stage D, and DVE free for the
        # s1m/pm/g chains.  s0 is only consumed mid-tile (D) and in E.
        s0 = actp.tile([128, HC, B], bf16, tag="s0", bufs=1, name=f"s0_{t}")
        for i in range(PAIRS):
            psl = slice(NP * i, NP * i + NP)
            q2 = tmpp.tile([128, NP, B], f32, tag="q", bufs=2,
                           name=f"q2_{t}_{i}")
            nc.vector.tensor_tensor(out=q2[:], in0=h1[:, psl, :],
                                    in1=h1[:, psl, :], op=mult)
            nc.vector.tensor_scalar(out=s0[:, psl, :], in0=q2[:], scalar1=-1.0,
                                    scalar2=1.0, op0=mult, op1=add)

        s1 = actp.tile([128, HC, B], f32r, tag="s1", bufs=1, name=f"s1_{t}")
        g1 = actp.tile([128, HC, B], f32r, tag="g1", bufs=1, name=f"g1_{t}")

        # ---- B+C: a2 -> s1; f1 -> g1 ----
        for i in range(PAIRS):
            pb = mmp.tile([128, NP, B], f32, tag="mm", name=f"pb_{t}_{i}")
            # pair 0 contracts k=6,7 last in both halves: h1 pairs 6,7
            # (riding in E(t-1)) finish latest.
            if i == 0:
                hk = ([(h, k) for h in range(NP) for k in range(6)]
                      + [(h, k) for h in range(NP) for k in range(6, HC)])
            else:
                hk = [(h, k) for h in range(NP) for k in range(HC)]
            for h, k in hk:
                nc.tensor.matmul(pb[:, h, :],
                                 w2t[:, k, (NP * i + h) * 128:
                                     (NP * i + h + 1) * 128],
                                 h1[:, k, :], start=(k == 0),
                                 stop=(k == HC - 1))
            pf = mmp.tile([128, NP, B], f32, tag="mm", name=f"pf_{t}_{i}")
            for h in range(NP):
                j = NP * i + h
                nc.tensor.matmul(pf[:, h, :], wf1t[:, j * 128:(j + 1) * 128],
                                 zut[:], start=True, stop=True)
            if i == 0 and tail is not None:
                tail()   # previous tile's last F-head pair + sout, hidden
                tail = None
            # s1 path needs only tanh', never tanh(a2):
            # 1 - tanh^2(a) = 4(sig - sig^2), sig = sigmoid(2a).
            # s1 array stores s1m = sig^2 - sig; the -4 is folded into
            # w2wn (host) and the pm fusion below.
            sg = tmpp.tile([128, NP, B], f32, tag="q", bufs=2,
                           name=f"sg_{t}_{i}")
            for h in range(NP):
                j = NP * i + h
                nc.scalar.activation(sg[:, h, :], pb[:, h, :], Sigmoid,
                                     bias=b2c[:, j:j + 1], scale=2.0)
            psl = slice(NP * i, NP * i + NP)
            nc.vector.scalar_tensor_tensor(out=s1[:, psl, :], in0=sg[:],
                                           scalar=-1.0, in1=sg[:],
                                           op0=add, op1=mult)
            th = tmpp.tile([128, NP, B], f32, tag="th", name=f"th_{t}_{i}")
            nc.scalar.activation(th[:], pf[:], Tanh)
            pm = tmpp.tile([128, NP, B], f32, tag="pm", name=f"pm_{t}_{i}")
            nc.vector.scalar_tensor_tensor(out=pm[:], in0=pf[:], scalar=-4.0,
                                           in1=s1[:, psl, :],
                                           op0=mult, op1=mult)
            nc.gpsimd.tensor_tensor(out=g1[:, psl, :], in0=th[:], in1=pm[:],
                                    op=add)

        # ---- D: pg = W2W.T @ s1m; ga1 = pg*s0; ps = W1.T @ ga1 ----
        # pairs 0 and 1 interleave their first 6 contraction chunks so the
        # j=6,7 matmuls land after the s1m[pair3] chain completes.
        ps = accp.tile([D2, B], f32, tag="acc", name=f"ps_{t}")

        def _pg_mms(pg, i, js):
            for h in range(NP):
                k = NP * i + h
                for j in js:
                    nc.tensor.matmul(pg[:, h, :],
                                     w2wn[:, j, k * 128:(k + 1) * 128],
                                     s1[:, j, :], start=(j == 0),
                                     stop=(j == HC - 1))

        def _ga1(pg, i):
            ga1 = grp.tile([128, NP, B], f32r, tag="gr", name=f"ga1_{t}_{i}")
            psl = slice(NP * i, NP * i + NP)
            nc.vector.tensor_tensor(out=ga1[:], in0=pg[:], in1=s0[:, psl, :],
                                    op=mult)
            return ga1

        def _dhead(ga1, i, start):
            for h in range(NP):
                k = NP * i + h
                nc.tensor.matmul(ps[:], w1n[:, k, :], ga1[:, h, :],
                                 start=(start and h == 0), stop=False)

        pg0 = mmp.tile([128, NP, B], f32, tag="mm", name=f"pg_{t}_0")
        pg1 = mmp.tile([128, NP, B], f32, tag="mm", name=f"pg_{t}_1")
        _pg_mms(pg0, 0, range(6))
        _pg_mms(pg1, 1, range(6))
        _pg_mms(pg0, 0, range(6, HC))
        ga1_0 = _ga1(pg0, 0)
        _pg_mms(pg1, 1, range(6, HC))
        ga1_1 = _ga1(pg1, 1)
        pg2 = mmp.tile([128, NP, B], f32, tag="mm", name=f"pg_{t}_2")
        _pg_mms(pg2, 2, range(HC))
        _dhead(ga1_0, 0, True)
        ga1_2 = _ga1(pg2, 2)
        pg3 = mmp.tile([128, NP, B], f32, tag="mm", name=f"pg_{t}_3")
        _pg_mms(pg3, 3, range(HC))
        _dhead(ga1_1, 1, False)
        ga1_3 = _ga1(pg3, 3)
        ga1_pen, ga1_last = ga1_2, ga1_3  # pair 2+3 heads deferred into E

        # ---- E: f2 -> g2 -> head accum; stage A(t+1) rides along ----
        if t + 1 < TILES:
            h1_n = actp.tile([128, HC, B], f32r, tag="h1", bufs=2,
                             name=f"h1_{t + 1}")
        g2_prev = None
        for i in range(PAIRS):
            if t + 1 < TILES:
                pa_n = _emit_A_mms(nc, mybir, mmp, w1t, zut_n, i, t + 1)
            pf2 = mmp.tile([128, NP, B], f32, tag="mm", name=f"pf2_{t}_{i}")
            for h in range(NP):
                j = NP * i + h
                for k in range(HC):
                    nc.tensor.matmul(pf2[:, h, :],
                                     wf2t[:, k, j * 128:(j + 1) * 128],
                                     g1[:, k, :], start=(k == 0),
                                     stop=(k == HC - 1))
            if i == 0:
                _dhead(ga1_pen, 2, False)
            if i == 1:
                _dhead(ga1_last, 3, False)
            th2 = tmpp.tile([128, NP, B], f32, tag="th", name=f"th2_{t}_{i}")
            for h in range(NP):
                j = NP * i + h
                nc.scalar.activation(th2[:, h, :], pf2[:, h, :], Tanh,
                                     bias=bf2c[:, j:j + 1])
            if t + 1 < TILES:
                # A-tanh after th2: th2 gates the F-head chain, h1 does not
                nc.scalar.activation(h1_n[:, NP * i:NP * i + NP, :], pa_n[:],
                                     Tanh)
            pm2 = tmpp.tile([128, NP, B], f32, tag="pm", name=f"pm2_{t}_{i}")
            psl = slice(NP * i, NP * i + NP)
            for h in range(NP):
                j = NP * i + h
                nc.vector.scalar_tensor_tensor(out=pm2[:, h, :],
                                               in0=pf2[:, h, :],
                                               scalar=bf2c[:, j:j + 1],
                                               in1=s0[:, j, :],
                                               op0=add, op1=mult)
            g2 = grp.tile([128, NP, B], f32r, tag="gr", name=f"g2_{t}_{i}")
            nc.gpsimd.tensor_tensor(out=g2[:], in0=th2[:], in1=pm2[:], op=add)
            if i >= 1:
                for h in range(NP):
                    j = NP * (i - 1) + h
                    nc.tensor.matmul(ps[:], wfft[:, j, :], g2_prev[:, h, :],
                                     start=False, stop=False)
            g2_prev = g2

        def _tail(ps=ps, g2_last=g2_prev, sl_t=sl_t, t=t):
            for h in range(NP):
                j = NP * (PAIRS - 1) + h
                nc.tensor.matmul(ps[:], wfft[:, j, :], g2_last[:, h, :],
                                 start=False, stop=(h == NP - 1))
            sout = iop.tile([D2, B], f32, tag="sout", bufs=1,
                            name=f"sout_{t}")
            nc.scalar.activation(sout[:], ps[:], Ident, bias=bffc[:, 0:1])
            nc.sync.dma_start(st_d[:, sl_t], sout[:])

        tail = _tail

        # roll state for next tile
        if t + 1 < TILES:
            zut = zut_n
            h1 = h1_n

    tail()


def _prep_inputs(t, z, W1, b1, W2, b2, Wh, bh, Wf1, bf1, Wf2, bf2, Wff, bff,
                 Wp, bp):
    f = np.float32
    z = np.asarray(z, f)
    u = np.tanh(z @ np.asarray(Wp, f).T + np.asarray(bp, f))
    ones = np.ones((z.shape[0], 1), f)
    zu1 = np.concatenate([z, ones, u], axis=1)   # [N, 21] rows: z, 1, u

    def c(x):
        return np.ascontiguousarray(np.asarray(x, f))

    W1 = np.asarray(W1, f); b1 = np.asarray(b1, f)
    W2 = np.asarray(W2, f); b2 = np.asarray(b2, f)
    Wh = np.asarray(Wh, f)
    Wf1 = np.asarray(Wf1, f); bf1 = np.asarray(bf1, f)
    Wf2 = np.asarray(Wf2, f); bf2 = np.asarray(bf2, f)
    Wff = np.asarray(Wff, f); bff = np.asarray(bff, f)

    # layer-A lhsT rows: z-features then the bias row
    w1t = np.concatenate([W1.T, b1[None, :]], axis=0)          # [17, H]
    # layer-C lhsT rows must match zu1 row order: z(16), ones, u(4)
    wf1t = np.concatenate([Wf1.T[:D2], bf1[None, :], Wf1.T[D2:]], axis=0)
    # s1 is stored as s1m = sig^2 - sig (sig = sigmoid(2*a2)); the true
    # tanh' = -4*s1m, so fold -4 into the backward weight product.
    w2w = -4.0 * W2 * Wh.reshape(H, 1)                          # [H, H]

    shared = {
        "w1t": c(w1t),
        "w1n": c(W1),
        "w2t": c(W2.T),
        "w2wn": c(w2w),
        "wf1t": c(wf1t),
        "wf2t": c(Wf2.T),
        "wfft": c(Wff.T),
        "b2c": c((2.0 * b2).reshape(HC, 128).T),
        "bf2c": c(bf2.reshape(HC, 128).T),
        "bffc": c(bff.reshape(D2, 1)),
    }
    in_maps = []
    for r in range(NCORES):
        m = dict(shared)
        m["zut"] = c(zu1[r * NSH:(r + 1) * NSH].T)
        in_maps.append(m)
    return in_maps


def _postprocess(results):
    outs = []
    for r in range(NCORES):
        s = results[r]["st"].T                    # [NSH, 16]
        outs.append(np.concatenate([s[:, DQ:], -s[:, :DQ]], axis=1))
    return np.ascontiguousarray(np.concatenate(outs, axis=0).astype(np.float32))


def kernel(**inputs):
    global _BUILT
    from concourse.bass_utils import run_bass_kernel_spmd

    if _BUILT is None:
        _BUILT = _build()
    in_maps = _prep_inputs(**inputs)
    res = run_bass_kernel_spmd(_BUILT, in_maps, list(range(NCORES)))
    return _postprocess(res.results)
